# revision 12
# baseline (speedup 1.0000x reference)
"""ALiBi bias kernel distributed across 8 TRN2 NeuronCores.

out[b,h,i,j] = scores[b,h,i,j] - slopes[h]*(pos_i-pos_j)
             = scores + negr_i + crow_j   (negr=-c*pos_i, crow=+c*pos_j, c=slope)

Memory-bound: fp8-e4m3 scores in, int8 (per-(b,h) scale) out -> 2 B/elem, ~34 MB
HBM traffic per core against the ~358 GB/s per-NC HBM ceiling (~97 us).

Two regions per core (rows of the local [4*2048, 2048] slab split between them):

 V-region (NV=32 blocks of [128 rows, 2048]): DVE scalar_tensor_tensor
   (scores + negr scalar + crow row) at 1x, ~2.35 us/block.

 T-region (NT=34 blocks of [122 rows, 2048]): ONE fused fp8 matmul per
   [*,512] tile. rhs partitions 0..121 = score rows, 122..127 = base-16
   digit vectors of pos (a, m, l, a/16, m/16, l/16 - all exact in e4m3);
   lhsT = [eye | per-row slope coefs (c1,c2 pieces, exponent-shifted so
   c2 stays in fp8-normal range)]. PSUM then holds scores + c*pos_j in one
   pass; ACT evicts PSUM -> int8 adding negr through the bias port.
   PE runs at 1.2 GHz here (HAM never warms), so the fused 4-mm block is
   ~2.1 us vs ~4.6 us for the eye+rank1 8-mm version.

 All out-DMAs ride the gpsimd (SWDGE) ring, in-DMAs the sync ring: keeps
 the ACT queue pure-evict and avoids head-of-line blocking of ins.

Digit-residual error ~ slope*2047/256 = 5.7 abs, int8 round = scale/2 = 5.8;
budget is 2e-2 * 1452 = 29."""

import numpy as np
import ml_dtypes

import concourse.bacc as bacc
import concourse.mybir as mybir
import concourse.tile as tile
from concourse.bass_utils import run_bass_kernel_spmd

NC = 8                 # NeuronCores
B, H, S = 2, 16, 2048  # scores: [B, H, S, S]
G = B * H              # 32 (b,h) slices
GP = G // NC           # 4 slices per core
P = 128                # SBUF partitions
F32 = mybir.dt.float32
F16 = mybir.dt.float16
F8 = mybir.dt.float8e4
I8 = mybir.dt.int8
NP_F8 = ml_dtypes.float8_e4m3

NV = 32                # V-region blocks of [128, S]
RV1 = NV * P // GP     # V rows per slice (1024)
TP = P - 6             # T-block score rows (122)
RT = GP * S - NV * P   # T-region rows total (4096)
NT = (RT + TP - 1) // TP          # 34 T blocks
RTP = NT * TP                     # padded T rows (4148)
KBV = (8, 8, 8, 8)     # V in-DMA group sizes
TSLOT = 8              # T blocks per ring buffer
TBUFS = 4
VBUFS = 4
N_MM = 512             # matmul N per PSUM bank


def _f8(x):
    return np.asarray(x, dtype=np.float32).astype(NP_F8)


def build(nv=None, nt=None):
    nv = NV if nv is None else nv
    nt = NT if nt is None else nt
    assert nv == NV and nt == NT
    nc = bacc.Bacc()
    sv_ext = nc.declare_dram_parameter("scores_v", [P, NV * S], F8, isOutput=False)
    st_ext = nc.declare_dram_parameter("scores_t", [TP, NT * S], F8, isOutput=False)
    crow_ext = nc.declare_dram_parameter("crow", [P, S], F16, isOutput=False)
    negv_ext = nc.declare_dram_parameter("negr_v", [P, NV], F32, isOutput=False)
    negt_ext = nc.declare_dram_parameter("negr_t", [P, NT], F32, isOutput=False)
    lhs_ext = nc.declare_dram_parameter("lhsT", [P, NT * P], F8, isOutput=False)
    aux_ext = nc.declare_dram_parameter("aux", [6, TSLOT * S], F8, isOutput=False)
    ov_ext = nc.declare_dram_parameter("out_v", [P, NV * S], I8, isOutput=True)
    ot_ext = nc.declare_dram_parameter("out_t", [TP, NT * S], I8, isOutput=True)

    with tile.TileContext(nc) as tc:
        with (
            tc.tile_pool(name="const", bufs=1) as cpool,
            tc.tile_pool(name="vout", bufs=6) as vpool,
            tc.tile_pool(name="tout", bufs=4) as tpool,
            tc.tile_pool(name="psum", bufs=2, space="PSUM") as ppool,
        ):
            crow_t = cpool.tile([P, S], F16, tag="crow")
            negv_t = cpool.tile([P, NV], F32, tag="negv")
            negt_t = cpool.tile([P, NT], F32, tag="negt")
            lhs_t = cpool.tile([P, NT * P], F8, tag="lhsT")
            nc.scalar.dma_start(crow_t[:, :], crow_ext[:, :])
            nc.scalar.dma_start(negv_t[:, :], negv_ext[:, :])
            nc.scalar.dma_start(negt_t[:, :], negt_ext[:, :])
            nc.scalar.dma_start(lhs_t[:, :], lhs_ext[:, :])

            # fixed-address in-tile rings; T rings carry the aux digit rows
            # in partitions 122..127, prefilled once per buffer
            vt = [cpool.tile([P, KBV[0] * S], F8, tag=f"vin{i}", name=f"vin{i}")
                  for i in range(VBUFS)]
            tt = [cpool.tile([P, TSLOT * S], F8, tag=f"tin{i}", name=f"tin{i}")
                  for i in range(TBUFS)]
            for i in range(TBUFS):
                nc.scalar.dma_start(tt[i][TP:P, :], aux_ext[:, :])

            # sync-ring fetch order: group 0 interleaves single-block V
            # pieces with single T blocks (fast ramp of all lanes); later
            # groups alternate [V group half][T slot run] coarsely
            def v_piece(g, f, sp):
                kb = KBV[g]
                step = kb * S // sp
                nc.sync.dma_start(
                    vt[g % VBUFS][:, f * step:(f + 1) * step],
                    sv_ext[:, g * 8 * S + f * step:g * 8 * S + (f + 1) * step])

            def t_piece(b):
                buf = tt[(b // TSLOT) % TBUFS]
                slot = b % TSLOT
                nc.sync.dma_start(
                    buf[0:TP, slot * S:(slot + 1) * S],
                    st_ext[:, b * S:(b + 1) * S])

            for k in range(8):                    # ramp: alternate 1-block units
                v_piece(0, k, 8)
                t_piece(k)
            for g in range(1, len(KBV)):
                v_piece(g, 0, 2)
                for b in range(g * 8, min(g * 8 + 8, nt)):
                    t_piece(b)
                v_piece(g, 1, 2)
            for b in range(len(KBV) * 8, nt):
                t_piece(b)

            # compute: interleave V and T blocks ~ in fetch order
            done_v, done_t = 0, 0
            while done_v < nv or done_t < nt:
                # one V block then one T block keeps DVE/PE/ACT queues fed
                if done_v < nv:
                    g, k = divmod(done_v, 8)
                    buf = vt[g % VBUFS]
                    o = vpool.tile([P, S], I8, tag="vo")
                    nc.vector.scalar_tensor_tensor(
                        o[:, :], buf[:, k * S:(k + 1) * S],
                        negv_t[:, done_v:done_v + 1], crow_t[:, 0:S],
                        op0=mybir.AluOpType.add, op1=mybir.AluOpType.add)
                    nc.gpsimd.dma_start(
                        ov_ext[:, done_v * S:(done_v + 1) * S], o[:, :])
                    done_v += 1
                if done_t < nt:
                    b = done_t
                    buf = tt[(b // TSLOT) % TBUFS]
                    slot = b % TSLOT
                    pt = ppool.tile([P, S], F32, tag="pt")
                    o = tpool.tile([TP, S], I8, tag="to")
                    for j in range(S // N_MM):
                        js = slice(j * N_MM, (j + 1) * N_MM)
                        nc.tensor.matmul(
                            pt[:, js], lhs_t[:, b * P:(b + 1) * P],
                            buf[:, slot * S + j * N_MM:slot * S + (j + 1) * N_MM],
                            start=True, stop=True)
                    nc.scalar.activation(
                        o[:, :], pt[0:TP, :],
                        mybir.ActivationFunctionType.Identity,
                        bias=negt_t[0:TP, b:b + 1], scale=1.0)
                    nc.gpsimd.dma_start(ot_ext[:, b * S:(b + 1) * S], o[:, :])
                    done_t += 1
    nc.compile()
    return nc


def make_scales(scores, slopes, positions, offset):
    """Per-(b,h) int8 scale: |out| <= slope*(pos range) + |scores|max."""
    slopes = np.asarray(slopes, dtype=np.float32).reshape(H)
    positions = np.asarray(positions, dtype=np.float32)
    pos = positions[:S] + np.float32(float(np.asarray(offset)))
    pr = float(pos.max() - pos.min())
    smax = float(np.abs(scores).max()) + 0.5
    slopes_g = np.broadcast_to(slopes[None, :], (B, H)).reshape(G)
    return ((slopes_g * pr + smax) / 126.0).astype(np.float32)


def _vrow_map():
    """global row (within core slab of GP*S rows) for V (p, n)."""
    p = np.arange(P)[:, None]
    n = np.arange(NV)[None, :]
    return (p // 32) * S + (p % 32) * NV + n      # [P, NV]


def _trow_map():
    """global row for T linear index q (0..RTP-1); -1 for pad."""
    q = np.arange(RTP)
    sl = np.minimum(q // RV1, GP - 1)              # slice id via 1024 rows each
    t = q % RV1
    rows = sl * S + RV1 + t                        # slice sl, rows RV1..2047
    rows[q >= RT] = -1
    return rows                                    # [RTP]


def make_in_maps(scores, slopes, positions, offset, scales):
    scores = np.asarray(scores, dtype=np.float32).reshape(G, S, S)
    slopes = np.asarray(slopes, dtype=np.float32).reshape(H)
    positions = np.asarray(positions, dtype=np.float32)
    pos = positions[:S] + np.float32(float(np.asarray(offset)))
    slopes_g = np.broadcast_to(slopes[None, :], (B, H)).reshape(G)
    pos_min = float(pos.min())
    posp = (pos - pos_min).astype(np.float64)      # >= 0, ints for arange
    # base-16 digits of pos' (exact in fp8 when pos' are ints < 4096)
    da = np.floor(posp / 256.0)
    dm = np.floor((posp - 256 * da) / 16.0)
    dl = posp - 256 * da - 16 * dm
    digits = np.stack([da, dm, dl, da / 16, dm / 16, dl / 16])  # [6, S]

    vmap = _vrow_map()                             # [P, NV]
    tmap = _trow_map()                             # [RTP]

    in_maps = []
    for c in range(NC):
        sl_loc = slopes_g[c * GP:(c + 1) * GP]              # [GP]
        inv_loc = (1.0 / scales[c * GP:(c + 1) * GP]).astype(np.float32)
        sc = scores[c * GP:(c + 1) * GP].reshape(GP * S, S)  # local slab
        inv_row = np.repeat(inv_loc, S)                      # [GP*S]
        sl_row = np.repeat(sl_loc, S)
        c_row = (sl_row * inv_row).astype(np.float32)        # slope*inv per row
        pos_row = np.tile(pos, GP)                           # pos_i per row

        sc_scaled = sc * inv_row[:, None]

        # ---- V region ----
        sv = sc_scaled[vmap.reshape(-1)].reshape(P, NV, S)
        scores_v = np.ascontiguousarray(sv.reshape(P, NV * S).astype(NP_F8))
        negr_v = (-c_row[vmap] * pos_row[vmap]).astype(np.float32)   # [P, NV]
        cp = c_row[vmap[:, 0]]                                       # [P]
        crow = (cp[:, None].astype(np.float32)
                * pos[None, :].astype(np.float32)).astype(np.float16)

        # ---- T region ----
        st = np.zeros((RTP, S), dtype=np.float32)
        valid = tmap >= 0
        st[valid] = sc_scaled[tmap[valid]]
        # partition-major: [TP, NT*S] so each in-DMA is 122 long segments
        scores_t = np.ascontiguousarray(
            st.reshape(NT, TP, S).transpose(1, 0, 2).reshape(TP, NT * S)
            .astype(NP_F8))
        c_q = np.zeros(RTP, dtype=np.float32)
        c_q[valid] = c_row[tmap[valid]]
        negr_t_q = np.zeros(RTP, dtype=np.float32)
        negr_t_q[valid] = c_row[tmap[valid]] * (pos_min - pos_row[tmap[valid]])
        # coef pieces: c1 = fp8(c), c2x = c - c1 (encoded exponent-shifted)
        c1 = _f8(c_q).astype(np.float32)
        c2x = (c_q - c1).astype(np.float32)
        coef = np.zeros((6, RTP), dtype=np.float32)
        coef[0] = c1 * 256.0
        coef[1] = c1 * 16.0
        coef[2] = c1
        coef[3] = c2x * 4096.0
        coef[4] = c2x * 256.0
        coef[5] = c2x * 16.0
        # lhsT[k, m] for block b: k<TP -> eye; k=TP+i -> coef[i, 122b+m]
        lhsT = np.zeros((P, NT, P), dtype=np.float32)
        for k in range(TP):
            lhsT[k, :, k] = 1.0
        for i in range(6):
            lhsT[TP + i, :, 0:TP] = coef[i].reshape(NT, TP)
        lhsT_f8 = np.ascontiguousarray(lhsT.reshape(P, NT * P).astype(NP_F8))
        negr_t = np.zeros((P, NT), dtype=np.float32)
        negr_t[0:TP, :] = negr_t_q.reshape(NT, TP).T

        aux = np.ascontiguousarray(
            np.tile(digits, (1, TSLOT)).astype(NP_F8))       # [6, TSLOT*S]

        in_maps.append({
            "scores_v": scores_v, "scores_t": scores_t, "crow": crow,
            "negr_v": negr_v, "negr_t": negr_t, "lhsT": lhsT_f8,
            "aux": aux,
        })
    return in_maps


def decode(res_list, scales):
    vmap = _vrow_map()
    tmap = _trow_map()
    valid = tmap >= 0
    outs = []
    for c in range(NC):
        slab = np.empty((GP * S, S), dtype=np.float32)
        ov = np.asarray(res_list[c]["out_v"]).astype(np.float32)
        ot = np.asarray(res_list[c]["out_t"]).astype(np.float32)
        ot = ot.reshape(TP, NT, S).transpose(1, 0, 2).reshape(RTP, S)
        slab[vmap.reshape(-1)] = ov.reshape(P, NV, S).reshape(P * NV, S)
        slab[tmap[valid]] = ot[valid]
        sc = scales[c * GP:(c + 1) * GP]
        slab = slab.reshape(GP, S, S) * sc[:, None, None]
        outs.append(slab)
    return np.concatenate(outs, axis=0).reshape(B, H, S, S)


def kernel(**inputs):
    scores = np.asarray(inputs["scores"])
    slopes = np.asarray(inputs["slopes"])
    positions = np.asarray(inputs["positions"])
    offset = inputs.get("offset", 0)
    scales = make_scales(scores, slopes, positions, offset)
    in_maps = make_in_maps(scores, slopes, positions, offset, scales)
    nc = build()
    res = run_bass_kernel_spmd(nc, in_maps, core_ids=list(range(NC)))
    return decode(res.results, scales)


# revision 13
# speedup vs baseline: 1.2446x; 1.2446x over previous
"""ALiBi bias kernel distributed across 8 TRN2 NeuronCores.

out[b,h,i,j] = scores[b,h,i,j] - slopes[h]*(pos_i-pos_j)
             = scores + negr_i + crow_j   (negr=-c*pos_i, crow=+c*pos_j, c=slope)

Memory-bound: fp8-e4m3 scores in, int8 (per-(b,h) scale) out -> 2 B/elem, ~34 MB
HBM traffic per core against the ~358 GB/s per-NC HBM ceiling (~97 us).

Two regions per core (rows of the local [4*2048, 2048] slab split between them):

 V-region (NV=32 blocks of [128 rows, 2048]): DVE scalar_tensor_tensor
   (scores + negr scalar + crow row) at 1x, ~2.35 us/block.

 T-region (NT=34 blocks of [122 rows, 2048]): ONE fused fp8 matmul per
   [*,512] tile. rhs partitions 0..121 = score rows, 122..127 = base-16
   digit vectors of pos (a, m, l, a/16, m/16, l/16 - all exact in e4m3);
   lhsT = [eye | per-row slope coefs (c1,c2 pieces, exponent-shifted so
   c2 stays in fp8-normal range)]. PSUM then holds scores + c*pos_j in one
   pass; ACT evicts PSUM -> int8 adding negr through the bias port.
   PE runs at 1.2 GHz here (HAM never warms), so the fused 4-mm block is
   ~2.1 us vs ~4.6 us for the eye+rank1 8-mm version.

 All out-DMAs ride the gpsimd (SWDGE) ring, in-DMAs the sync ring: keeps
 the ACT queue pure-evict and avoids head-of-line blocking of ins.

Digit-residual error ~ slope*2047/256 = 5.7 abs, int8 round = scale/2 = 5.8;
budget is 2e-2 * 1452 = 29."""

import numpy as np
import ml_dtypes

import concourse.bacc as bacc
import concourse.mybir as mybir
import concourse.tile as tile
from concourse.bass_utils import run_bass_kernel_spmd

NC = 8                 # NeuronCores
B, H, S = 2, 16, 2048  # scores: [B, H, S, S]
G = B * H              # 32 (b,h) slices
GP = G // NC           # 4 slices per core
P = 128                # SBUF partitions
F32 = mybir.dt.float32
F16 = mybir.dt.float16
F8 = mybir.dt.float8e4
I8 = mybir.dt.int8
NP_F8 = ml_dtypes.float8_e4m3

NV = 32                # V-region blocks of [128, S]
RV1 = NV * P // GP     # V rows per slice (1024)
TP = P - 6             # T-block score rows (122)
RT = GP * S - NV * P   # T-region rows total (4096)
NT = (RT + TP - 1) // TP          # 34 T blocks
RTP = NT * TP                     # padded T rows (4148)
KBV = (8, 8, 8, 8)     # V in-DMA group sizes
TSLOT = 8              # T blocks per ring buffer
TBUFS = 4
VBUFS = 4
N_MM = 512             # matmul N per PSUM bank


def _f8(x):
    return np.asarray(x, dtype=np.float32).astype(NP_F8)


def build(nv=None, nt=None):
    nv = NV if nv is None else nv
    nt = NT if nt is None else nt
    assert nv == NV and nt == NT
    nc = bacc.Bacc()
    sv_ext = nc.declare_dram_parameter("scores_v", [P, NV * S], F8, isOutput=False)
    st_ext = nc.declare_dram_parameter("scores_t", [TP, NT * S], F8, isOutput=False)
    crow_ext = nc.declare_dram_parameter("crow", [P, S], F16, isOutput=False)
    negv_ext = nc.declare_dram_parameter("negr_v", [P, NV], F32, isOutput=False)
    negt_ext = nc.declare_dram_parameter("negr_t", [P, NT], F32, isOutput=False)
    lhs_ext = nc.declare_dram_parameter("lhsT", [P, NT * P], F8, isOutput=False)
    aux_ext = nc.declare_dram_parameter("aux", [6, TSLOT * S], F8, isOutput=False)
    ov_ext = nc.declare_dram_parameter("out_v", [P, NV * S], I8, isOutput=True)
    ot_ext = nc.declare_dram_parameter("out_t", [TP, NT * S], I8, isOutput=True)

    with tile.TileContext(nc) as tc:
        with (
            tc.tile_pool(name="const", bufs=1) as cpool,
            tc.tile_pool(name="vout", bufs=6) as vpool,
            tc.tile_pool(name="tout", bufs=4) as tpool,
            tc.tile_pool(name="psum", bufs=2, space="PSUM") as ppool,
        ):
            crow_t = cpool.tile([P, S], F16, tag="crow")
            negv_t = cpool.tile([P, NV], F32, tag="negv")
            negt_t = cpool.tile([P, NT], F32, tag="negt")
            lhs_t = cpool.tile([P, NT * P], F8, tag="lhsT")
            nc.scalar.dma_start(crow_t[:, :], crow_ext[:, :])
            nc.scalar.dma_start(negv_t[:, :], negv_ext[:, :])
            nc.scalar.dma_start(negt_t[:, :], negt_ext[:, :])
            nc.scalar.dma_start(lhs_t[:, :], lhs_ext[:, :])

            # fixed-address in-tile rings; T rings carry the aux digit rows
            # in partitions 122..127, prefilled once per buffer
            vt = [cpool.tile([P, KBV[0] * S], F8, tag=f"vin{i}", name=f"vin{i}")
                  for i in range(VBUFS)]
            tt = [cpool.tile([P, TSLOT * S], F8, tag=f"tin{i}", name=f"tin{i}")
                  for i in range(TBUFS)]
            for i in range(TBUFS):
                nc.scalar.dma_start(tt[i][TP:P, :], aux_ext[:, :])

            # sync-ring fetch order: group 0 interleaves single-block V
            # pieces with single T blocks (fast ramp of all lanes); later
            # groups alternate [V group half][T slot run] coarsely
            def v_piece(g, f, sp):
                kb = KBV[g]
                step = kb * S // sp
                nc.sync.dma_start(
                    vt[g % VBUFS][:, f * step:(f + 1) * step],
                    sv_ext[:, g * 8 * S + f * step:g * 8 * S + (f + 1) * step])

            def t_piece(b):
                buf = tt[(b // TSLOT) % TBUFS]
                slot = b % TSLOT
                nc.sync.dma_start(
                    buf[0:TP, slot * S:(slot + 1) * S],
                    st_ext[:, b * S:(b + 1) * S])

            def v_block(v):
                g, k = divmod(v, 8)
                buf = vt[g % VBUFS]
                o = vpool.tile([P, S], I8, tag="vo")
                nc.vector.scalar_tensor_tensor(
                    o[:, :], buf[:, k * S:(k + 1) * S],
                    negv_t[:, v:v + 1], crow_t[:, 0:S],
                    op0=mybir.AluOpType.add, op1=mybir.AluOpType.add)
                nc.gpsimd.dma_start(ov_ext[:, v * S:(v + 1) * S], o[:, :])

            def t_block(b):
                buf = tt[(b // TSLOT) % TBUFS]
                slot = b % TSLOT
                pt = ppool.tile([P, S], F32, tag="pt")
                o = tpool.tile([TP, S], I8, tag="to")
                for j in range(S // N_MM):
                    js = slice(j * N_MM, (j + 1) * N_MM)
                    nc.tensor.matmul(
                        pt[:, js], lhs_t[:, b * P:(b + 1) * P],
                        buf[:, slot * S + j * N_MM:slot * S + (j + 1) * N_MM],
                        start=True, stop=True)
                nc.scalar.activation(
                    o[:, :], pt[0:TP, :],
                    mybir.ActivationFunctionType.Identity,
                    bias=negt_t[0:TP, b:b + 1], scale=1.0)
                nc.gpsimd.dma_start(ot_ext[:, b * S:(b + 1) * S], o[:, :])

            # Pipelined emission: [ins for unit][compute+outs for unit].
            # Tile multiplexes DMA completions onto 8 rotating sem lanes and
            # each DMA waits its lane's previous user; emitting all ins up
            # front interleaves compute-gated outs into the lane sequence
            # ahead of in-DMAs, which serializes the whole kernel (measured
            # 3x). Group-wise emission keeps lane order = pipeline order.
            # Unit 0 is block-granular so all lanes ramp within ~2 us.
            for k in range(4):
                v_piece(0, 2 * k, 8)
                t_piece(k)
                v_piece(0, 2 * k + 1, 8)
                v_block(k)
                t_block(k)
            for k in range(4, 8):
                t_piece(k)
                v_block(k)
                t_block(k)
            for g in range(1, len(KBV)):
                v_piece(g, 0, 2)
                for b in range(g * 8, g * 8 + 4):
                    t_piece(b)
                v_piece(g, 1, 2)
                for b in range(g * 8 + 4, min(g * 8 + 8, nt)):
                    t_piece(b)
                for k in range(8):
                    v_block(g * 8 + k)
                    if g * 8 + k < nt:
                        t_block(g * 8 + k)
            for b in range(len(KBV) * 8, nt):
                t_piece(b)
            for b in range(len(KBV) * 8, nt):
                t_block(b)
    nc.compile()
    return nc


def make_scales(scores, slopes, positions, offset):
    """Per-(b,h) int8 scale: |out| <= slope*(pos range) + |scores|max."""
    slopes = np.asarray(slopes, dtype=np.float32).reshape(H)
    positions = np.asarray(positions, dtype=np.float32)
    pos = positions[:S] + np.float32(float(np.asarray(offset)))
    pr = float(pos.max() - pos.min())
    smax = float(np.abs(scores).max()) + 0.5
    slopes_g = np.broadcast_to(slopes[None, :], (B, H)).reshape(G)
    return ((slopes_g * pr + smax) / 126.0).astype(np.float32)


def _vrow_map():
    """global row (within core slab of GP*S rows) for V (p, n)."""
    p = np.arange(P)[:, None]
    n = np.arange(NV)[None, :]
    return (p // 32) * S + (p % 32) * NV + n      # [P, NV]


def _trow_map():
    """global row for T linear index q (0..RTP-1); -1 for pad."""
    q = np.arange(RTP)
    sl = np.minimum(q // RV1, GP - 1)              # slice id via 1024 rows each
    t = q % RV1
    rows = sl * S + RV1 + t                        # slice sl, rows RV1..2047
    rows[q >= RT] = -1
    return rows                                    # [RTP]


def make_in_maps(scores, slopes, positions, offset, scales):
    scores = np.asarray(scores, dtype=np.float32).reshape(G, S, S)
    slopes = np.asarray(slopes, dtype=np.float32).reshape(H)
    positions = np.asarray(positions, dtype=np.float32)
    pos = positions[:S] + np.float32(float(np.asarray(offset)))
    slopes_g = np.broadcast_to(slopes[None, :], (B, H)).reshape(G)
    pos_min = float(pos.min())
    posp = (pos - pos_min).astype(np.float64)      # >= 0, ints for arange
    # base-16 digits of pos' (exact in fp8 when pos' are ints < 4096)
    da = np.floor(posp / 256.0)
    dm = np.floor((posp - 256 * da) / 16.0)
    dl = posp - 256 * da - 16 * dm
    digits = np.stack([da, dm, dl, da / 16, dm / 16, dl / 16])  # [6, S]

    vmap = _vrow_map()                             # [P, NV]
    tmap = _trow_map()                             # [RTP]

    in_maps = []
    for c in range(NC):
        sl_loc = slopes_g[c * GP:(c + 1) * GP]              # [GP]
        inv_loc = (1.0 / scales[c * GP:(c + 1) * GP]).astype(np.float32)
        sc = scores[c * GP:(c + 1) * GP].reshape(GP * S, S)  # local slab
        inv_row = np.repeat(inv_loc, S)                      # [GP*S]
        sl_row = np.repeat(sl_loc, S)
        c_row = (sl_row * inv_row).astype(np.float32)        # slope*inv per row
        pos_row = np.tile(pos, GP)                           # pos_i per row

        sc_scaled = sc * inv_row[:, None]

        # ---- V region ----
        sv = sc_scaled[vmap.reshape(-1)].reshape(P, NV, S)
        scores_v = np.ascontiguousarray(sv.reshape(P, NV * S).astype(NP_F8))
        negr_v = (-c_row[vmap] * pos_row[vmap]).astype(np.float32)   # [P, NV]
        cp = c_row[vmap[:, 0]]                                       # [P]
        crow = (cp[:, None].astype(np.float32)
                * pos[None, :].astype(np.float32)).astype(np.float16)

        # ---- T region ----
        st = np.zeros((RTP, S), dtype=np.float32)
        valid = tmap >= 0
        st[valid] = sc_scaled[tmap[valid]]
        # partition-major: [TP, NT*S] so each in-DMA is 122 long segments
        scores_t = np.ascontiguousarray(
            st.reshape(NT, TP, S).transpose(1, 0, 2).reshape(TP, NT * S)
            .astype(NP_F8))
        c_q = np.zeros(RTP, dtype=np.float32)
        c_q[valid] = c_row[tmap[valid]]
        negr_t_q = np.zeros(RTP, dtype=np.float32)
        negr_t_q[valid] = c_row[tmap[valid]] * (pos_min - pos_row[tmap[valid]])
        # coef pieces: c1 = fp8(c), c2x = c - c1 (encoded exponent-shifted)
        c1 = _f8(c_q).astype(np.float32)
        c2x = (c_q - c1).astype(np.float32)
        coef = np.zeros((6, RTP), dtype=np.float32)
        coef[0] = c1 * 256.0
        coef[1] = c1 * 16.0
        coef[2] = c1
        coef[3] = c2x * 4096.0
        coef[4] = c2x * 256.0
        coef[5] = c2x * 16.0
        # lhsT[k, m] for block b: k<TP -> eye; k=TP+i -> coef[i, 122b+m]
        lhsT = np.zeros((P, NT, P), dtype=np.float32)
        for k in range(TP):
            lhsT[k, :, k] = 1.0
        for i in range(6):
            lhsT[TP + i, :, 0:TP] = coef[i].reshape(NT, TP)
        lhsT_f8 = np.ascontiguousarray(lhsT.reshape(P, NT * P).astype(NP_F8))
        negr_t = np.zeros((P, NT), dtype=np.float32)
        negr_t[0:TP, :] = negr_t_q.reshape(NT, TP).T

        aux = np.ascontiguousarray(
            np.tile(digits, (1, TSLOT)).astype(NP_F8))       # [6, TSLOT*S]

        in_maps.append({
            "scores_v": scores_v, "scores_t": scores_t, "crow": crow,
            "negr_v": negr_v, "negr_t": negr_t, "lhsT": lhsT_f8,
            "aux": aux,
        })
    return in_maps


def decode(res_list, scales):
    vmap = _vrow_map()
    tmap = _trow_map()
    valid = tmap >= 0
    outs = []
    for c in range(NC):
        slab = np.empty((GP * S, S), dtype=np.float32)
        ov = np.asarray(res_list[c]["out_v"]).astype(np.float32)
        ot = np.asarray(res_list[c]["out_t"]).astype(np.float32)
        ot = ot.reshape(TP, NT, S).transpose(1, 0, 2).reshape(RTP, S)
        slab[vmap.reshape(-1)] = ov.reshape(P, NV, S).reshape(P * NV, S)
        slab[tmap[valid]] = ot[valid]
        sc = scales[c * GP:(c + 1) * GP]
        slab = slab.reshape(GP, S, S) * sc[:, None, None]
        outs.append(slab)
    return np.concatenate(outs, axis=0).reshape(B, H, S, S)


def kernel(**inputs):
    scores = np.asarray(inputs["scores"])
    slopes = np.asarray(inputs["slopes"])
    positions = np.asarray(inputs["positions"])
    offset = inputs.get("offset", 0)
    scales = make_scales(scores, slopes, positions, offset)
    in_maps = make_in_maps(scores, slopes, positions, offset, scales)
    nc = build()
    res = run_bass_kernel_spmd(nc, in_maps, core_ids=list(range(NC)))
    return decode(res.results, scales)
